# revision 28
# baseline (speedup 1.0000x reference)
import sys

sys.path.insert(0, "/opt/trn_rl_repo")

import ml_dtypes
import numpy as np

import bass_rust
import concourse.bass as bass
import concourse.mybir as mybir
import concourse.tile as tile
from concourse import bass_utils
from concourse.tile import ScopedClock

B, T, C = 4, 2048, 1024
H, HD = 16, 64
HPC = 8
GC = HPC * HD
QB = 512
KBLK = 128
NQC = T // QB
NKT = T // KBLK
KT = C // 128

F32 = mybir.dt.float32
BF16 = mybir.dt.bfloat16
BF16NP = ml_dtypes.bfloat16


_MAX_WAITS = 1


def _split_multi_waits(nc: bass.Bass) -> None:
    eng_by_type = nc.engines

    n_es = [0]

    def make_nop(engine_type, wait):
        eng = eng_by_type[engine_type]
        if engine_type == mybir.EngineType.Pool:
            inst = mybir.InstEventSemaphore(
                name=f"I-wsplit-es-{n_es[0]}", ins=[], outs=[]
            )
            n_es[0] += 1
            inst.engine = engine_type
            inst.sync_info = bass_rust.SyncInfo(on_wait=[wait], on_update=[])
            return inst
        binst = eng.nop(hint="wsplit", nofuse=True)
        cur = nc.cur_bb.bb
        insts = list(cur.instructions)
        assert insts and insts[-1] is binst.ins
        cur.instructions = insts[:-1]
        binst.ins.sync_info = bass_rust.SyncInfo(on_wait=[wait], on_update=[])
        return binst.ins

    for f in nc.m.functions:
        for bb in f.blocks:
            changed = False
            new_insts = []
            for inst in bb.instructions:
                si = inst.sync_info
                waits = list(si.on_wait) if si is not None and si.on_wait else []
                if len(waits) > _MAX_WAITS:
                    for w in waits[:-_MAX_WAITS]:
                        new_insts.append(make_nop(inst.engine, w))
                    si.on_wait = waits[-_MAX_WAITS:]
                    changed = True
                new_insts.append(inst)
            if changed:
                bb.instructions = new_insts


def _drain_and_barrier_split(self, tick_clock, wait_clock):
    nc = self.nc
    drain_inst = nc.sync.drain()
    wait_clock.add_sem_waits(
        drain_inst.ins, ScopedClock({None: tick_clock.global_clock})
    )
    nc.all_engine_barrier()
    assert self.sems is not None
    popped = nc._tile_sem_poison_stack.pop()
    assert popped is self._sem_poison
    nc.clear_and_free_semaphores(list(self.sems.allocated().values()))
    nc.all_engine_barrier()
    _split_multi_waits(nc)


tile.TileContext._drain_and_barrier = _drain_and_barrier_split


def _act_recip(nc: bass.Bass, out_ap, in_ap):
    return nc.scalar.add_instruction(
        mybir.InstActivation(
            name=nc.get_next_instruction_name(),
            func=mybir.ActivationFunctionType.Reciprocal,
            ins=[
                nc.scalar.lower_ap(in_ap),
                mybir.ImmediateValue(dtype=F32, value=0.0),
                mybir.ImmediateValue(dtype=F32, value=1.0),
                mybir.ImmediateValue(dtype=F32, value=0.0),
            ],
            outs=[nc.scalar.lower_ap(out_ap)],
        )
    )


def build_nc(with_bias: bool) -> bass.Bass:
    nc = bass.Bass("TRN2", target_bir_lowering=False)

    xT = nc.declare_dram_parameter("xT", [C, T], BF16, isOutput=False)
    wqk = nc.declare_dram_parameter("wqk", [C, 2 * GC], BF16, isOutput=False)
    wv = nc.declare_dram_parameter("wv", [C, GC], BF16, isOutput=False)
    wp = nc.declare_dram_parameter("wp", [GC, C], BF16, isOutput=False)
    maskp = nc.declare_dram_parameter("mask", [128, 4 * QB], BF16, isOutput=False)
    if with_bias:
        bqk = nc.declare_dram_parameter("bqk", [1, 2 * GC], BF16, isOutput=False)
        bv = nc.declare_dram_parameter("bv", [1, GC], BF16, isOutput=False)
    out = nc.declare_dram_parameter("out", [T, C], F32, isOutput=True)

    with tile.TileContext(nc) as tc:
        with (
            tc.tile_pool(name="singles", bufs=1) as singles,
            tc.tile_pool(name="exp", bufs=7) as exp_pool,
            tc.tile_pool(name="small", bufs=4) as small_pool,
            tc.tile_pool(name="recipp", bufs=2) as recip_pool,
            tc.tile_pool(name="ytu", bufs=2) as ytu_pool,
            tc.tile_pool(name="outsb", bufs=2) as out_pool,
            tc.tile_pool(name="dram", bufs=4, space="DRAM") as dram_pool,
            tc.tile_pool(name="ps", bufs=2, space="PSUM") as ps_pool,
            tc.tile_pool(name="ps_att", bufs=2, space="PSUM") as ps_att_pool,
            tc.tile_pool(name="ps_y", bufs=2, space="PSUM") as ps_y_pool,
        ):
            xT_sb = singles.tile([128, KT, T], BF16, tag="xT")
            wqk_sb = singles.tile([128, KT, 2 * GC], BF16, tag="wqk")
            wv_sb = singles.tile([128, KT, GC], BF16, tag="wv")
            wp_sb = singles.tile([128, 4, C], BF16, tag="wp")
            mask_sb = singles.tile([128, 4 * QB], BF16, tag="mask")
            qkT_sbs = [
                singles.tile([128, T], BF16, tag=f"qkT{mt}", name=f"qkT{mt}")
                for mt in range(8)
            ]
            vv_sb = singles.tile([128, HPC, NKT, HD + 1], BF16, tag="vv")
            yTn_sbs = [
                singles.tile([128, T], BF16, tag=f"yTn{ct}", name=f"yTn{ct}")
                for ct in range(4)
            ]

            nc.sync.dma_start(
                out=xT_sb[:], in_=xT.rearrange("(kt p) t -> p kt t", p=128)
            )
            nc.sync.dma_start(
                out=wqk_sb[:], in_=wqk.rearrange("(kt p) m -> p kt m", p=128)
            )
            nc.sync.dma_start(
                out=wv_sb[:], in_=wv.rearrange("(kt p) m -> p kt m", p=128)
            )
            nc.sync.dma_start(
                out=wp_sb[:], in_=wp.rearrange("(ct p) m -> p ct m", p=128)
            )
            nc.sync.dma_start(out=mask_sb[:], in_=maskp[:, :])
            if with_bias:
                bqk_sb = singles.tile([1, 2 * GC], BF16, tag="bqk")
                bv_sb = singles.tile([1, GC], BF16, tag="bv")
                ones_sb = singles.tile([1, T], BF16, tag="ones")
                nc.sync.dma_start(out=bqk_sb[:], in_=bqk[:, :])
                nc.sync.dma_start(out=bv_sb[:], in_=bv[:, :])
                nc.vector.memset(ones_sb[:], 1.0)

            nc.vector.memset(vv_sb[:, :, :, HD], 1.0)

            for tt in range(NKT):
                ps = ps_pool.tile([128, QB], F32, tag="ps")
                for kt in range(KT):
                    nc.tensor.matmul(
                        ps[:],
                        lhsT=xT_sb[:, kt, tt * 128 : (tt + 1) * 128],
                        rhs=wv_sb[:, kt, :],
                        start=(kt == 0),
                        stop=(kt == KT - 1 and not with_bias),
                    )
                if with_bias:
                    nc.tensor.matmul(
                        ps[:],
                        lhsT=ones_sb[0:1, tt * 128 : (tt + 1) * 128],
                        rhs=bv_sb[0:1, :],
                        start=False,
                        stop=True,
                    )
                nc.vector.tensor_copy(
                    vv_sb[:, :, tt, 0:HD],
                    ps[:].rearrange("p (h d) -> p h d", h=HPC),
                )

            def emit_qkT_unit(mt, ntc):
                ps = ps_pool.tile([128, QB], F32, tag="ps", name="ps")
                for kt in range(KT):
                    nc.tensor.matmul(
                        ps[:],
                        lhsT=wqk_sb[:, kt, mt * 128 : (mt + 1) * 128],
                        rhs=xT_sb[:, kt, ntc * QB : (ntc + 1) * QB],
                        start=(kt == 0),
                        stop=(kt == KT - 1 and not with_bias),
                    )
                if with_bias:
                    nc.tensor.matmul(
                        ps[:],
                        lhsT=bqk_sb[0:1, mt * 128 : (mt + 1) * 128],
                        rhs=ones_sb[0:1, ntc * QB : (ntc + 1) * QB],
                        start=False,
                        stop=True,
                    )
                nc.vector.tensor_copy(
                    qkT_sbs[mt][:, ntc * QB : (ntc + 1) * QB], ps[:]
                )

            for mt in (0, 4):
                for ntc in range(NQC):
                    emit_qkT_unit(mt, ntc)
            qkT_fill = [
                (mt, ntc) for mt in (1, 5, 2, 6, 3, 7) for ntc in range(NQC)
            ]
            fst = {"pos": 0, "ps": None, "kt": 0}

            def emit_filler_chunk():
                if fst["ps"] is None:
                    if fst["pos"] >= len(qkT_fill):
                        return
                    fst["mt"], fst["ntc"] = qkT_fill[fst["pos"]]
                    fst["pos"] += 1
                    fst["ps"] = ps_pool.tile([128, QB], F32, tag="ps", name="ps")
                    fst["kt"] = 0
                mt, ntc = fst["mt"], fst["ntc"]
                ps = fst["ps"]
                for _ in range(2):
                    kt = fst["kt"]
                    nc.tensor.matmul(
                        ps[:],
                        lhsT=wqk_sb[:, kt, mt * 128 : (mt + 1) * 128],
                        rhs=xT_sb[:, kt, ntc * QB : (ntc + 1) * QB],
                        start=(kt == 0),
                        stop=(kt == KT - 1 and not with_bias),
                    )
                    fst["kt"] += 1
                    if fst["kt"] == KT:
                        if with_bias:
                            nc.tensor.matmul(
                                ps[:],
                                lhsT=bqk_sb[0:1, mt * 128 : (mt + 1) * 128],
                                rhs=ones_sb[0:1, ntc * QB : (ntc + 1) * QB],
                                start=False,
                                stop=True,
                            )
                        nc.vector.tensor_copy(
                            qkT_sbs[mt][:, ntc * QB : (ntc + 1) * QB], ps[:]
                        )
                        fst["ps"] = None
                        break

            for h in range(HPC):
                prt = 64 * (h % 2)
                qt = qkT_sbs[h // 2]
                kt_sb = qkT_sbs[4 + h // 2]
                vv = vv_sb[:, h]
                ytu = ytu_pool.tile([HD + 1, NQC, QB], BF16, tag="ytu")
                for qc in range(NQC):
                    nkb = 4 * (qc + 1)
                    exp_ts = []
                    def blk_off(kb):
                        m = kb - 4 * qc
                        return 128 * m if m > 0 else 0

                    for kb2 in range(0, nkb, 2):
                        ps_att = ps_att_pool.tile([128, 2 * QB], F32, tag="ps_att")
                        for u in (0, 1):
                            kb = kb2 + u
                            nc.tensor.matmul(
                                ps_att[:, u * QB : (u + 1) * QB],
                                lhsT=kt_sb[prt : prt + 64, kb * 128 : (kb + 1) * 128],
                                rhs=qt[prt : prt + 64, qc * QB : (qc + 1) * QB],
                                start=True,
                                stop=True,
                            )
                            emit_filler_chunk()
                        exp_t = exp_pool.tile([128, 2 * QB], BF16, tag="exp")
                        nc.scalar.activation(
                            exp_t[:],
                            ps_att[:],
                            mybir.ActivationFunctionType.Exp,
                            scale=0.125,
                        )
                        for u in (0, 1):
                            kb = kb2 + u
                            if kb >= 4 * qc:
                                m = kb - 4 * qc
                                w = 128 * (m + 1)
                                nc.vector.tensor_mul(
                                    exp_t[:, u * QB : u * QB + w],
                                    exp_t[:, u * QB : u * QB + w],
                                    mask_sb[:, m * QB : m * QB + w],
                                )
                        exp_ts.append(exp_t)
                    ps_y = ps_y_pool.tile([HD + 1, QB], F32, tag="ps_y")
                    for kb in range(nkb):
                        nc.tensor.matmul(
                            ps_y[:],
                            lhsT=vv_sb[:, h, kb, :],
                            rhs=exp_ts[kb // 2][
                                :, (kb % 2) * QB : (kb % 2 + 1) * QB
                            ],
                            start=(kb == 0),
                            stop=(kb == nkb - 1),
                        )
                        emit_filler_chunk()
                    nc.vector.tensor_copy(ytu[:, qc, :], ps_y[:])
                recip = recip_pool.tile([1, NQC * QB], BF16, tag="recip")
                _act_recip(nc, recip[:], ytu[HD : HD + 1, :, :])
                recip_dram = dram_pool.tile([1, NQC * QB], BF16, tag="recip_dram")
                nc.sync.dma_start(out=recip_dram[:], in_=recip[:])
                for qc in range(NQC):
                    bcast = small_pool.tile([64, QB], BF16, tag="bcast")
                    nc.sync.dma_start(
                        out=bcast[:],
                        in_=recip_dram[0:1, qc * QB : (qc + 1) * QB].to_broadcast(
                            (64, QB)
                        ),
                    )
                    nc.vector.tensor_mul(
                        yTn_sbs[h // 2][prt : prt + 64, qc * QB : (qc + 1) * QB],
                        ytu[0:HD, qc, :],
                        bcast[:],
                    )

            for tt in range(NKT):
                out_sb = out_pool.tile([128, C], F32, tag="out_sb")
                for nt2 in range(2):
                    ps = ps_pool.tile([128, QB], F32, tag="ps")
                    for ct in range(4):
                        nc.tensor.matmul(
                            ps[:],
                            lhsT=yTn_sbs[ct][:, tt * 128 : (tt + 1) * 128],
                            rhs=wp_sb[:, ct, nt2 * QB : (nt2 + 1) * QB],
                            start=(ct == 0),
                            stop=(ct == 3),
                        )
                    nc.vector.tensor_copy(
                        out_sb[:, nt2 * QB : (nt2 + 1) * QB], ps[:]
                    )
                nc.sync.dma_start(
                    out=out[tt * 128 : (tt + 1) * 128, :], in_=out_sb[:]
                )

    return nc


def _make_mask() -> np.ndarray:
    p = np.arange(128)[:, None]
    i = np.arange(QB)[None, :]
    blocks = [(p + 128 * m <= i) for m in range(4)]
    return np.concatenate(blocks, axis=1).astype(BF16NP)


_NC_CACHE: dict[bool, bass.Bass] = {}


def kernel(x, w_qkv, b_qkv, w_proj, b_proj):
    x = np.asarray(x, dtype=np.float32)
    w_qkv = np.asarray(w_qkv, dtype=np.float32)
    b_qkv = np.asarray(b_qkv, dtype=np.float32)
    w_proj = np.asarray(w_proj, dtype=np.float32)
    b_proj = np.asarray(b_proj, dtype=np.float32)

    with_bias = bool(np.any(b_qkv))
    if with_bias not in _NC_CACHE:
        _NC_CACHE[with_bias] = build_nc(with_bias)
    nc = _NC_CACHE[with_bias]

    mask = _make_mask()
    in_maps = []
    for c in range(8):
        b, g = c // 2, c % 2
        cols = slice(g * GC, (g + 1) * GC)
        m = {
            "xT": np.ascontiguousarray(x[b].T).astype(BF16NP),
            "wqk": np.concatenate(
                [w_qkv[:, cols], w_qkv[:, C:][:, cols]], axis=1
            ).astype(BF16NP),
            "wv": np.ascontiguousarray(w_qkv[:, 2 * C :][:, cols]).astype(BF16NP),
            "wp": np.ascontiguousarray(w_proj[cols, :]).astype(BF16NP),
            "mask": mask,
        }
        if with_bias:
            m["bqk"] = np.concatenate([b_qkv[cols], b_qkv[C:][cols]])[None, :].astype(
                BF16NP
            )
            m["bv"] = b_qkv[2 * C :][cols][None, :].astype(BF16NP)
        in_maps.append(m)

    res = bass_utils.run_bass_kernel_spmd(nc, in_maps, core_ids=list(range(8)))

    out = np.empty((B, T, C), dtype=np.float32)
    for b in range(B):
        out[b] = res.results[2 * b]["out"] + res.results[2 * b + 1]["out"] + b_proj
    return out


# revision 30
# speedup vs baseline: 1.2497x; 1.2497x over previous
import sys

sys.path.insert(0, "/opt/trn_rl_repo")

import ml_dtypes
import numpy as np

import bass_rust
import concourse.bass as bass
import concourse.mybir as mybir
import concourse.tile as tile
from concourse import bass_utils
from concourse.tile import ScopedClock

B, T, C = 4, 2048, 1024
H, HD = 16, 64
HPC = 8
GC = HPC * HD
QB = 512
KBLK = 128
NQC = T // QB
NKT = T // KBLK
KT = C // 128

F32 = mybir.dt.float32
BF16 = mybir.dt.bfloat16
BF16NP = ml_dtypes.bfloat16


_MAX_WAITS = 1


def _split_multi_waits(nc: bass.Bass) -> None:
    eng_by_type = nc.engines

    n_es = [0]

    def make_nop(engine_type, wait):
        eng = eng_by_type[engine_type]
        if engine_type == mybir.EngineType.Pool:
            inst = mybir.InstEventSemaphore(
                name=f"I-wsplit-es-{n_es[0]}", ins=[], outs=[]
            )
            n_es[0] += 1
            inst.engine = engine_type
            inst.sync_info = bass_rust.SyncInfo(on_wait=[wait], on_update=[])
            return inst
        binst = eng.nop(hint="wsplit", nofuse=True)
        cur = nc.cur_bb.bb
        insts = list(cur.instructions)
        assert insts and insts[-1] is binst.ins
        cur.instructions = insts[:-1]
        binst.ins.sync_info = bass_rust.SyncInfo(on_wait=[wait], on_update=[])
        return binst.ins

    for f in nc.m.functions:
        for bb in f.blocks:
            changed = False
            new_insts = []
            for inst in bb.instructions:
                si = inst.sync_info
                waits = list(si.on_wait) if si is not None and si.on_wait else []
                if len(waits) > _MAX_WAITS:
                    for w in waits[:-_MAX_WAITS]:
                        new_insts.append(make_nop(inst.engine, w))
                    si.on_wait = waits[-_MAX_WAITS:]
                    changed = True
                new_insts.append(inst)
            if changed:
                bb.instructions = new_insts


def _drain_and_barrier_split(self, tick_clock, wait_clock):
    nc = self.nc
    drain_inst = nc.sync.drain()
    wait_clock.add_sem_waits(
        drain_inst.ins, ScopedClock({None: tick_clock.global_clock})
    )
    nc.all_engine_barrier()
    assert self.sems is not None
    popped = nc._tile_sem_poison_stack.pop()
    assert popped is self._sem_poison
    nc.clear_and_free_semaphores(list(self.sems.allocated().values()))
    nc.all_engine_barrier()
    _split_multi_waits(nc)


tile.TileContext._drain_and_barrier = _drain_and_barrier_split


def _act_recip(nc: bass.Bass, out_ap, in_ap):
    return nc.scalar.add_instruction(
        mybir.InstActivation(
            name=nc.get_next_instruction_name(),
            func=mybir.ActivationFunctionType.Reciprocal,
            ins=[
                nc.scalar.lower_ap(in_ap),
                mybir.ImmediateValue(dtype=F32, value=0.0),
                mybir.ImmediateValue(dtype=F32, value=1.0),
                mybir.ImmediateValue(dtype=F32, value=0.0),
            ],
            outs=[nc.scalar.lower_ap(out_ap)],
        )
    )


def build_nc(with_bias: bool) -> bass.Bass:
    nc = bass.Bass("TRN2", target_bir_lowering=False)

    xT = nc.declare_dram_parameter("xT", [C, T], BF16, isOutput=False)
    wqk = nc.declare_dram_parameter("wqk", [C, 2 * GC], BF16, isOutput=False)
    wv = nc.declare_dram_parameter("wv", [C, GC], BF16, isOutput=False)
    wp = nc.declare_dram_parameter("wp", [GC, C], BF16, isOutput=False)
    maskp = nc.declare_dram_parameter("mask", [128, 4 * QB], BF16, isOutput=False)
    if with_bias:
        bqk = nc.declare_dram_parameter("bqk", [1, 2 * GC], BF16, isOutput=False)
        bv = nc.declare_dram_parameter("bv", [1, GC], BF16, isOutput=False)
    out = nc.declare_dram_parameter("out", [T, C], F32, isOutput=True)

    with tile.TileContext(nc) as tc:
        with (
            tc.tile_pool(name="singles", bufs=1) as singles,
            tc.tile_pool(name="exp", bufs=7) as exp_pool,
            tc.tile_pool(name="small", bufs=4) as small_pool,
            tc.tile_pool(name="recipp", bufs=2) as recip_pool,
            tc.tile_pool(name="ytu", bufs=2) as ytu_pool,
            tc.tile_pool(name="outsb", bufs=2) as out_pool,
            tc.tile_pool(name="dram", bufs=4, space="DRAM") as dram_pool,
            tc.tile_pool(name="ps", bufs=2, space="PSUM") as ps_pool,
            tc.tile_pool(name="ps_att", bufs=2, space="PSUM") as ps_att_pool,
            tc.tile_pool(name="ps_y", bufs=2, space="PSUM") as ps_y_pool,
        ):
            xT_sb = singles.tile([128, KT, T], BF16, tag="xT")
            wqk_sb = singles.tile([128, KT, 2 * GC], BF16, tag="wqk")
            wv_sb = singles.tile([128, KT, GC], BF16, tag="wv")
            wp_sb = singles.tile([128, 4, C], BF16, tag="wp")
            mask_sb = singles.tile([128, 4 * QB], BF16, tag="mask")
            qkT_sbs = [
                singles.tile([128, T], BF16, tag=f"qkT{mt}", name=f"qkT{mt}")
                for mt in range(8)
            ]
            vv_sb = singles.tile([128, HPC, NKT, HD + 1], BF16, tag="vv")
            yTn_sbs = [
                singles.tile([128, T], BF16, tag=f"yTn{ct}", name=f"yTn{ct}")
                for ct in range(4)
            ]

            nc.sync.dma_start(
                out=xT_sb[:], in_=xT.rearrange("(kt p) t -> p kt t", p=128)
            )
            nc.sync.dma_start(
                out=wqk_sb[:], in_=wqk.rearrange("(kt p) m -> p kt m", p=128)
            )
            nc.sync.dma_start(
                out=wv_sb[:], in_=wv.rearrange("(kt p) m -> p kt m", p=128)
            )
            nc.sync.dma_start(
                out=wp_sb[:], in_=wp.rearrange("(ct p) m -> p ct m", p=128)
            )
            nc.sync.dma_start(out=mask_sb[:], in_=maskp[:, :])
            if with_bias:
                bqk_sb = singles.tile([1, 2 * GC], BF16, tag="bqk")
                bv_sb = singles.tile([1, GC], BF16, tag="bv")
                ones_sb = singles.tile([1, T], BF16, tag="ones")
                nc.sync.dma_start(out=bqk_sb[:], in_=bqk[:, :])
                nc.sync.dma_start(out=bv_sb[:], in_=bv[:, :])
                nc.vector.memset(ones_sb[:], 1.0)

            nc.vector.memset(vv_sb[:, :, :, HD], 1.0)

            for tt in range(NKT):
                ps = ps_pool.tile([128, QB], F32, tag="ps")
                for kt in range(KT):
                    nc.tensor.matmul(
                        ps[:],
                        lhsT=xT_sb[:, kt, tt * 128 : (tt + 1) * 128],
                        rhs=wv_sb[:, kt, :],
                        start=(kt == 0),
                        stop=(kt == KT - 1 and not with_bias),
                    )
                if with_bias:
                    nc.tensor.matmul(
                        ps[:],
                        lhsT=ones_sb[0:1, tt * 128 : (tt + 1) * 128],
                        rhs=bv_sb[0:1, :],
                        start=False,
                        stop=True,
                    )
                nc.vector.tensor_copy(
                    vv_sb[:, :, tt, 0:HD],
                    ps[:].rearrange("p (h d) -> p h d", h=HPC),
                )

            def emit_qkT_unit(mt, ntc):
                ps = ps_pool.tile([128, QB], F32, tag="ps", name="ps")
                for kt in range(KT):
                    nc.tensor.matmul(
                        ps[:],
                        lhsT=wqk_sb[:, kt, mt * 128 : (mt + 1) * 128],
                        rhs=xT_sb[:, kt, ntc * QB : (ntc + 1) * QB],
                        start=(kt == 0),
                        stop=(kt == KT - 1 and not with_bias),
                    )
                if with_bias:
                    nc.tensor.matmul(
                        ps[:],
                        lhsT=bqk_sb[0:1, mt * 128 : (mt + 1) * 128],
                        rhs=ones_sb[0:1, ntc * QB : (ntc + 1) * QB],
                        start=False,
                        stop=True,
                    )
                nc.vector.tensor_copy(
                    qkT_sbs[mt][:, ntc * QB : (ntc + 1) * QB], ps[:]
                )

            for mt in (0, 4):
                for ntc in range(NQC):
                    emit_qkT_unit(mt, ntc)
            qkT_fill = [
                (mt, ntc) for mt in (1, 5, 2, 6, 3, 7) for ntc in range(NQC)
            ]
            fill_i = [0]

            def emit_filler():
                if fill_i[0] < len(qkT_fill):
                    emit_qkT_unit(*qkT_fill[fill_i[0]])
                    fill_i[0] += 1

            for h in range(HPC):
                prt = 64 * (h % 2)
                qt = qkT_sbs[h // 2]
                kt_sb = qkT_sbs[4 + h // 2]
                vv = vv_sb[:, h]
                ytu = ytu_pool.tile([HD + 1, NQC, QB], BF16, tag="ytu")
                for qc in range(NQC):
                    nkb = 4 * (qc + 1)
                    exp_ts = []
                    def blk_off(kb):
                        m = kb - 4 * qc
                        return 128 * m if m > 0 else 0

                    for kb2 in range(0, nkb, 2):
                        ps_att = ps_att_pool.tile([128, 2 * QB], F32, tag="ps_att")
                        for u in (0, 1):
                            kb = kb2 + u
                            nc.tensor.matmul(
                                ps_att[:, u * QB : (u + 1) * QB],
                                lhsT=kt_sb[prt : prt + 64, kb * 128 : (kb + 1) * 128],
                                rhs=qt[prt : prt + 64, qc * QB : (qc + 1) * QB],
                                start=True,
                                stop=True,
                            )
                        exp_t = exp_pool.tile([128, 2 * QB], BF16, tag="exp")
                        nc.scalar.activation(
                            exp_t[:],
                            ps_att[:],
                            mybir.ActivationFunctionType.Exp,
                            scale=0.125,
                        )
                        for u in (0, 1):
                            kb = kb2 + u
                            if kb >= 4 * qc:
                                m = kb - 4 * qc
                                w = 128 * (m + 1)
                                nc.vector.tensor_mul(
                                    exp_t[:, u * QB : u * QB + w],
                                    exp_t[:, u * QB : u * QB + w],
                                    mask_sb[:, m * QB : m * QB + w],
                                )
                        exp_ts.append(exp_t)
                    ps_y = ps_y_pool.tile([HD + 1, QB], F32, tag="ps_y")
                    for kb in range(nkb):
                        nc.tensor.matmul(
                            ps_y[:],
                            lhsT=vv_sb[:, h, kb, :],
                            rhs=exp_ts[kb // 2][
                                :, (kb % 2) * QB : (kb % 2 + 1) * QB
                            ],
                            start=(kb == 0),
                            stop=(kb == nkb - 1),
                        )
                    nc.vector.tensor_copy(ytu[:, qc, :], ps_y[:])
                    emit_filler()
                recip = recip_pool.tile([1, NQC * QB], BF16, tag="recip")
                _act_recip(nc, recip[:], ytu[HD : HD + 1, :, :])
                recip_dram = dram_pool.tile([1, NQC * QB], BF16, tag="recip_dram")
                nc.sync.dma_start(out=recip_dram[:], in_=recip[:])
                for qc in range(NQC):
                    bcast = small_pool.tile([64, QB], BF16, tag="bcast")
                    nc.sync.dma_start(
                        out=bcast[:],
                        in_=recip_dram[0:1, qc * QB : (qc + 1) * QB].to_broadcast(
                            (64, QB)
                        ),
                    )
                    nc.vector.tensor_mul(
                        yTn_sbs[h // 2][prt : prt + 64, qc * QB : (qc + 1) * QB],
                        ytu[0:HD, qc, :],
                        bcast[:],
                    )

            for tt in range(NKT):
                out_sb = out_pool.tile([128, C], F32, tag="out_sb")
                for nt2 in range(2):
                    ps = ps_pool.tile([128, QB], F32, tag="ps")
                    for ct in range(4):
                        nc.tensor.matmul(
                            ps[:],
                            lhsT=yTn_sbs[ct][:, tt * 128 : (tt + 1) * 128],
                            rhs=wp_sb[:, ct, nt2 * QB : (nt2 + 1) * QB],
                            start=(ct == 0),
                            stop=(ct == 3),
                        )
                    nc.vector.tensor_copy(
                        out_sb[:, nt2 * QB : (nt2 + 1) * QB], ps[:]
                    )
                nc.sync.dma_start(
                    out=out[tt * 128 : (tt + 1) * 128, :], in_=out_sb[:]
                )

    return nc


def _make_mask() -> np.ndarray:
    p = np.arange(128)[:, None]
    i = np.arange(QB)[None, :]
    blocks = [(p + 128 * m <= i) for m in range(4)]
    return np.concatenate(blocks, axis=1).astype(BF16NP)


_NC_CACHE: dict[bool, bass.Bass] = {}


def kernel(x, w_qkv, b_qkv, w_proj, b_proj):
    x = np.asarray(x, dtype=np.float32)
    w_qkv = np.asarray(w_qkv, dtype=np.float32)
    b_qkv = np.asarray(b_qkv, dtype=np.float32)
    w_proj = np.asarray(w_proj, dtype=np.float32)
    b_proj = np.asarray(b_proj, dtype=np.float32)

    with_bias = bool(np.any(b_qkv))
    if with_bias not in _NC_CACHE:
        _NC_CACHE[with_bias] = build_nc(with_bias)
    nc = _NC_CACHE[with_bias]

    mask = _make_mask()
    in_maps = []
    for c in range(8):
        b, g = c // 2, c % 2
        cols = slice(g * GC, (g + 1) * GC)
        m = {
            "xT": np.ascontiguousarray(x[b].T).astype(BF16NP),
            "wqk": np.concatenate(
                [w_qkv[:, cols], w_qkv[:, C:][:, cols]], axis=1
            ).astype(BF16NP),
            "wv": np.ascontiguousarray(w_qkv[:, 2 * C :][:, cols]).astype(BF16NP),
            "wp": np.ascontiguousarray(w_proj[cols, :]).astype(BF16NP),
            "mask": mask,
        }
        if with_bias:
            m["bqk"] = np.concatenate([b_qkv[cols], b_qkv[C:][cols]])[None, :].astype(
                BF16NP
            )
            m["bv"] = b_qkv[2 * C :][cols][None, :].astype(BF16NP)
        in_maps.append(m)

    res = bass_utils.run_bass_kernel_spmd(nc, in_maps, core_ids=list(range(8)))

    out = np.empty((B, T, C), dtype=np.float32)
    for b in range(B):
        out[b] = res.results[2 * b]["out"] + res.results[2 * b + 1]["out"] + b_proj
    return out
